# revision 17
# baseline (speedup 1.0000x reference)
"""DeltaEncoder (delta -> BatchNorm(eval) -> Linear(1,O) -> LIF scan over O) on 8 TRN2 cores.

Strategy (pure data parallel over batch B=32 -> 4 per core):
  * Host folds BN (eval) + Linear(1,O) + the 1/TAU charge factor into per-o
    scalars  A[o] = inv*w[o]/TAU,  C[o] = ((bn_b - mu*inv)*w[o] + b[o])/TAU,
    so the per-step membrane charge is  h = (1-1/TAU)*v + (delta*A[o] + C[o]).
  * Host pre-transposes the per-core input to [(b f), t] so the device sees
    elements as [128 partitions = (b%2, f), free = (b//2, t)] with t contiguous.
  * Device computes delta along t once, then runs the 64-step LIF scan with a
    single fused custom DVE instruction per step:
        h' = (h < 1) * (h * (1-1/TAU)) + (delta * A_o + C_o)
    (state update incl. hard reset + charge, one pass at 1 elem/lane/cycle).
    Steps within a group write adjacent NFREE-slots of one group h-tile.
  * Spikes for a whole group (up to 8 o-steps) are extracted by ONE scalar-
    engine op  s = sigmoid(2^100*(h-1))  (saturates to exactly 0.0/1.0) with
    fp8_e4m3 output -- 1 byte/spike -- then stored with one contiguous DMA
    per group into a blocked DRAM layout [p=(b1,f), o, g, t]; the host
    permutes back to [B, O, F, T] (fp8 -> f32 via byte != 0, exact).
Output: float32 spikes [B, O, F, T].
"""

import numpy as np

# problem shapes (hardcoded per contract)
_B, _T, _F, _O = 32, 512, 64, 64
_NC = 8
_BL = _B // _NC          # 4 batches per core
_G = (_BL * _F) // 128   # 2 free-dim groups of 128 (b,f) rows
_P = 128
_TAU = 2.0
_EPS = 1e-5

_LIF_OP_NAME = "LIF_STEP_ANT_RT"

# which engine extracts spikes: "gpsimd" | "dve" | "act"
SPIKE_ENGINE = "act"
STEPS_PER_DMA = 8
# explicit group-size override (list summing to _O); None -> derived taper
GROUPS_OVERRIDE = None
# number of spike-extract ops per store group (1 = one ACT op per group)
EXTRACT_SPLIT = 4
# use the raw (no TileContext) builder for the production act/fp8 path:
# same dataflow, but no per-op self-semaphores on the DVE chain (engine
# program order already guarantees same-engine RAW; each op's first read
# trails the previous op's first write by a full 1024-cycle stream)
RAW = True
# raw builder: extract the final group's last slice on DVE (is_ge, 2x mode)
# instead of ACT — shortens the kernel tail
RAW_TAIL_DVE = False
# raw builder: emit the epilogue barrier + semaphore clear (needed for safe
# NEFF re-execution; disable only for timing bisection)
RAW_EPILOGUE = True
# engine for the small memsets (d t=0 column, sigmoid bias)
MEMSET_ENGINE = "gpsimd"
# split the input DMA + delta into this many chunks along g (pipeline rampup)
INPUT_SPLIT = 1
HPOOL_BUFS = 3
SPOOL_BUFS = 3
# bake the folded per-o scalars into instruction immediates (saves ~7us/call
# at the cost of a content-keyed NEFF compile on first use)
USE_IMM = True
# spike store dtype on device: "fp8" (float8_e4m3, 1 byte -- 0.0/1.0 exact,
# host widens via byte != 0) | "bf16" | "f32".
SPIKE_DTYPE = "fp8"

_MODULE_CACHE = {}


def _register_lif_op():
    """Register the fused LIF-step custom DVE op (idempotent)."""
    import concourse.dve_ops as dve_ops
    from concourse.dve_spec import (
        C0, C1, C2, One, Spec, Src0, Src1, _has_src1, lower,
    )
    from concourse.dve_uop import DveOpSpec

    for op in dve_ops.OPS:
        if op.name == _LIF_OP_NAME:
            return op

    def _ref(in0, in1, s0, s1, imm2):
        in0 = np.asarray(in0, np.float32)
        in1 = np.asarray(in1, np.float32)
        keep = (in0 < np.float32(1.0)).astype(np.float32)
        return (
            keep * (in0 * np.float32(imm2))
            + (in1 * np.float32(s0) + np.float32(s1))
        ).astype(np.float32)

    body = (Src0 < One) * (Src0 * C2) + (Src1 * C0 + C1)
    spec = Spec(body=body, reference=_ref)

    row = dve_ops._CUSTOM_DVE_ROW_BASE + len(dve_ops.OPS)
    assert row < 0x20, "no free custom-DVE opcode rows"
    shas = {}
    for ver in ("v3", "v4"):
        uops = lower(spec, ver=ver)
        shas[ver] = DveOpSpec(
            name=_LIF_OP_NAME, opcode=row, uops=uops, rd1_en=_has_src1(spec)
        ).sha(ver)

    op = dve_ops.DveOp(_LIF_OP_NAME, spec, subdim=False, uops_sha=shas)
    dve_ops.OPS.append(op)
    dve_ops._SUB_OPCODE_FOR_NAME[op.name] = row
    dve_ops.CUSTOM_DVE_SPECS[op.name] = spec
    return op


def _spike_groups():
    """Store/extract group sizes along o: SPD-step groups, tapered at the
    end to shrink the kernel tail (small final ACT + DMA)."""
    if GROUPS_OVERRIDE is not None:
        assert sum(GROUPS_OVERRIDE) == _O
        return list(GROUPS_OVERRIDE)
    groups = []
    rem = _O
    while rem > 2 * STEPS_PER_DMA:
        groups.append(STEPS_PER_DMA)
        rem -= STEPS_PER_DMA
    while rem > 0:
        g_sz = rem if rem <= 2 else max(2, rem // 2)
        groups.append(g_sz)
        rem -= g_sz
    return groups


def _build_module(
    spike_engine: str, reps: int = 1, variant: str = "full", imm_coefs=None
):
    """Build the Bass/Tile module (one NeuronCore program, SPMD across 8).

    reps > 1 wraps the whole computation in a device-side loop -- used only
    for differential wall-clock timing (output is rewritten identically).
    variant: "full" | "scanonly" (no spikes/stores) | "nostore" (no DMA out)
             | "noscan" (one memset h + spikes/stores only) -- bench-only.
    """
    import concourse.bacc as bacc
    import concourse.mybir as mybir
    from concourse.tile import TileContext

    lif_op = _register_lif_op()

    nc = bacc.Bacc(
        "TRN2",
        target_bir_lowering=False,
        debug=False,
        enable_asserts=False,
        num_devices=_NC,
    )
    f32 = mybir.dt.float32

    NFREE = _G * _T          # 1024
    groups = _spike_groups()

    x_in = nc.dram_tensor("x_bft", [_BL * _F, _T], f32, kind="ExternalInput").ap()
    a_in = nc.dram_tensor("a_coef", [_P, _O], f32, kind="ExternalInput").ap()
    c_in = nc.dram_tensor("c_coef", [_P, _O], f32, kind="ExternalInput").ap()
    if SPIKE_DTYPE == "fp8":
        sdt = mybir.dt.float8e4
    elif SPIKE_DTYPE == "bf16":
        sdt = mybir.dt.bfloat16
    else:
        sdt = f32
    # blocked layout: [p=(b1,f), o, g, t]; host permutes to [b=2g+b1, o, f, t]
    out = nc.dram_tensor(
        "spikes", [_P, _O, _G, _T], sdt, kind="ExternalOutput"
    ).ap()
    out2 = out.rearrange("p o g t -> p (o g t)")  # [128, O*NFREE], contiguous rows

    with TileContext(nc) as tc:
        with (
            tc.tile_pool(name="const", bufs=1) as cpool,
            tc.tile_pool(name="xd", bufs=1) as xpool,
            tc.tile_pool(name="state", bufs=HPOOL_BUFS) as hpool,
            tc.tile_pool(name="spk", bufs=SPOOL_BUFS) as spool,
        ):

            def body():
                if variant == "empty":
                    z_t = cpool.tile([_P, 1], f32, tag="z")
                    nc.vector.memset(z_t[:], 0.0)
                    return
                a_t = c_t = None
                if imm_coefs is None:
                    a_t = cpool.tile([_P, _O], f32, tag="a")
                    c_t = cpool.tile([_P, _O], f32, tag="c")
                    nc.sync.dma_start(out=a_t[:], in_=a_in[:])
                    nc.sync.dma_start(out=c_t[:], in_=c_in[:])

                x_t = xpool.tile([_P, NFREE], f32, tag="x")
                d_t = xpool.tile([_P, NFREE], f32, tag="d")
                x3 = x_t[:].rearrange("p (g t) -> p g t", g=_G)
                d3 = d_t[:].rearrange("p (g t) -> p g t", g=_G)
                x_src = x_in.rearrange("(g p) t -> p g t", p=_P)
                mset = nc.gpsimd if MEMSET_ENGINE == "gpsimd" else nc.vector
                # delta along t: d[...,0] = 0 ; d[...,1:] = x[...,1:] - x[...,:-1]
                # (input DMA + sub optionally chunked along g for faster rampup)
                if INPUT_SPLIT <= 1:
                    nc.sync.dma_start(out=x3, in_=x_src)
                    mset.memset(d3[:, :, 0:1], 0.0)
                    nc.vector.tensor_sub(
                        out=d3[:, :, 1:_T], in0=x3[:, :, 1:_T],
                        in1=x3[:, :, 0 : _T - 1],
                    )
                else:
                    for gi in range(_G):
                        nc.sync.dma_start(
                            out=x3[:, gi : gi + 1], in_=x_src[:, gi : gi + 1]
                        )
                    mset.memset(d3[:, :, 0:1], 0.0)
                    for gi in range(_G):
                        nc.vector.tensor_sub(
                            out=d3[:, gi : gi + 1, 1:_T],
                            in0=x3[:, gi : gi + 1, 1:_T],
                            in1=x3[:, gi : gi + 1, 0 : _T - 1],
                        )

                sigb = None
                if spike_engine == "act" and variant != "scanonly":
                    sigb = cpool.tile([_P, 1], f32, tag="sigb")
                    mset.memset(sigb[:], -(2.0**100))
                h_zero = None
                if variant == "noscan":
                    # one static h tile reused by every group's extraction
                    h_zero = cpool.tile([_P, STEPS_PER_DMA * NFREE], f32, tag="hz")
                    nc.vector.memset(h_zero[:], 0.0)

                decay = 1.0 - 1.0 / _TAU
                o_base = 0
                h_half = None   # AP of the previous step's h slot
                for g_sz in groups:
                    if variant != "noscan":
                        h_g = hpool.tile([_P, g_sz * NFREE], f32, tag="h")
                        for oi in range(g_sz):
                            o = o_base + oi
                            out_ap = h_g[:, oi * NFREE : (oi + 1) * NFREE]
                            if imm_coefs is not None:
                                s0o, s1o = float(imm_coefs[0][o]), float(imm_coefs[1][o])
                            else:
                                s0o, s1o = a_t[:, o : o + 1], c_t[:, o : o + 1]
                            if o == 0:
                                # v=0: h_0 = d*A_0 + C_0 (2x-mode tensor_scalar,
                                # replaces state memset + first custom op)
                                nc.vector.tensor_scalar(
                                    out_ap,
                                    d_t[:],
                                    s0o,
                                    s1o,
                                    mybir.AluOpType.mult,
                                    mybir.AluOpType.add,
                                )
                            else:
                                nc.vector._custom_dve(
                                    lif_op,
                                    out=out_ap,
                                    in0=h_half,
                                    in1=d_t[:],
                                    s0=s0o,
                                    s1=s1o,
                                    imm2=decay,
                                )
                            h_half = out_ap
                    else:
                        h_g = h_zero
                    if variant == "scanonly":
                        o_base += g_sz
                        continue
                    s_mega = spool.tile([_P, g_sz * NFREE], sdt, tag="s")
                    nsp = min(EXTRACT_SPLIT, g_sz)
                    bounds = [g_sz * i // nsp for i in range(nsp + 1)]
                    for b0, b1 in zip(bounds[:-1], bounds[1:]):
                        s_ap = s_mega[:, b0 * NFREE : b1 * NFREE]
                        h_ap = h_g[:, b0 * NFREE : b1 * NFREE]
                        if spike_engine == "gpsimd":
                            nc.gpsimd.tensor_single_scalar(
                                s_ap, h_ap, 1.0, mybir.AluOpType.is_ge
                            )
                        elif spike_engine == "act":
                            # sigmoid(2^100*(h-1)) saturates to exactly 0/1
                            nc.scalar.activation(
                                s_ap,
                                h_ap,
                                mybir.ActivationFunctionType.Sigmoid,
                                bias=sigb[:],
                                scale=2.0**100,
                            )
                        else:
                            nc.vector.tensor_single_scalar(
                                s_ap, h_ap, 1.0, mybir.AluOpType.is_ge
                            )
                    if variant != "nostore":
                        lo = o_base * NFREE
                        hi = (o_base + g_sz) * NFREE
                        nc.sync.dma_start(out=out2[:, lo:hi], in_=s_mega[:])
                    o_base += g_sz

            if reps == 1:
                body()
            else:
                with tc.For_i(0, reps, 1):
                    body()

    nc.finalize()
    return nc


def _build_module_raw(imm_coefs):
    """No-Tile production builder (act engine, fp8 spikes, imm coefs).

    Identical dataflow to _build_module(variant="full"), but semaphores only
    on true cross-engine edges:
      s_x    input DMA done            -> DVE delta
      s_dve  DVE scan progress         -> ACT extract slices
      s_act  ACT extracts done         -> SP store DMA + DVE h-ring reuse
      s_dma[j] store DMA (s-slot j)    -> ACT s-ring slot reuse
    The DVE scan chain itself carries no waits: the engine is in-order and
    each op's first read trails the previous op's first write by a full
    1024-element stream (~1024 cycles >> SBUF write latency).
    """
    import contextlib

    import concourse.bacc as bacc
    import concourse.mybir as mybir

    assert imm_coefs is not None
    lif_op = _register_lif_op()

    nc = bacc.Bacc(
        "TRN2",
        target_bir_lowering=False,
        debug=False,
        enable_asserts=False,
        num_devices=_NC,
    )
    f32 = mybir.dt.float32
    sdt = {"fp8": mybir.dt.float8e4, "bf16": mybir.dt.bfloat16,
           "f32": f32}[SPIKE_DTYPE]

    NFREE = _G * _T          # 1024
    groups = _spike_groups()
    ngrp = len(groups)
    NRING = 3                # h / s ring depth
    HMAX = max(groups)

    x_in = nc.dram_tensor("x_bft", [_BL * _F, _T], f32, kind="ExternalInput").ap()
    out = nc.dram_tensor(
        "spikes", [_P, _O, _G, _T], sdt, kind="ExternalOutput"
    ).ap()
    out2 = out.rearrange("p o g t -> p (o g t)")

    s_x = nc.alloc_semaphore("s_x")
    s_dve = nc.alloc_semaphore("s_dve")
    s_act = nc.alloc_semaphore("s_act")
    s_dma = [nc.alloc_semaphore(f"s_dma{j}") for j in range(NRING)]

    with contextlib.ExitStack() as stack:
        xt = stack.enter_context(nc.sbuf_tensor("x", [_P, NFREE], f32))
        dt_ = stack.enter_context(nc.sbuf_tensor("d", [_P, NFREE], f32))
        sgt = stack.enter_context(nc.sbuf_tensor("sigb", [_P, 1], f32))
        hts = [stack.enter_context(
                   nc.sbuf_tensor(f"h{j}", [_P, HMAX * NFREE], f32))
               for j in range(NRING)]
        sts = [stack.enter_context(
                   nc.sbuf_tensor(f"s{j}", [_P, HMAX * NFREE], sdt))
               for j in range(NRING)]
        x_t, d_t, sigb = xt.ap(), dt_.ap(), sgt.ap()
        h_bufs = [t.ap() for t in hts]
        s_bufs = [t.ap() for t in sts]

        x3 = x_t.rearrange("p (g t) -> p g t", g=_G)
        d3 = d_t.rearrange("p (g t) -> p g t", g=_G)

        nc.sync.dma_start(
            out=x3, in_=x_in.rearrange("(g p) t -> p g t", p=_P)
        ).then_inc(s_x, 16)

        # DVE prologue: sigmoid bias, delta (waits for the input DMA)
        nc.vector.memset(sigb, -(2.0**100))
        nc.vector.wait_ge(s_x, 16)
        nc.vector.tensor_sub(
            out=d3[:, :, 1:_T], in0=x3[:, :, 1:_T], in1=x3[:, :, 0 : _T - 1]
        )
        nc.vector.memset(d3[:, :, 0:1], 0.0)

        # static schedule bookkeeping
        decay = 1.0 - 1.0 / _TAU
        dve_incs = 0          # s_dve increments issued so far
        act_incs = 0          # s_act increments issued so far
        dma_cnt = [0] * NRING  # store DMAs issued per s-ring slot
        # per-group precomputed: extract slice bounds
        o_base = 0
        h_half = None
        acts_after_group = []
        for gi, g_sz in enumerate(groups):
            h_g = h_bufs[gi % NRING]
            s_g = s_bufs[gi % NRING]
            nsp = min(EXTRACT_SPLIT, g_sz)
            bounds = [g_sz * i // nsp for i in range(nsp + 1)]
            if gi >= NRING:
                # h-ring reuse: ACT must have finished reading group gi-NRING
                nc.vector.wait_ge(s_act, acts_after_group[gi - NRING])
            for oi in range(g_sz):
                o = o_base + oi
                out_ap = h_g[:, oi * NFREE : (oi + 1) * NFREE]
                s0o = float(imm_coefs[0][o])
                s1o = float(imm_coefs[1][o])
                if o == 0:
                    ins = nc.vector.tensor_scalar(
                        out_ap, d_t, s0o, s1o,
                        mybir.AluOpType.mult, mybir.AluOpType.add,
                    )
                else:
                    ins = nc.vector._custom_dve(
                        lif_op, out=out_ap, in0=h_half, in1=d_t,
                        s0=s0o, s1=s1o, imm2=decay,
                    )
                h_half = out_ap
                if oi + 1 in bounds:
                    ins.then_inc(s_dve, 1)
                    dve_incs += 1
            # ACT extraction for this group, slice by slice
            for si, (b0, b1) in enumerate(zip(bounds[:-1], bounds[1:])):
                on_dve = (
                    RAW_TAIL_DVE and gi == ngrp - 1 and si == nsp - 1
                )
                if on_dve:
                    # final slice: extract on DVE right after the last LIF
                    # step (program order, no sem hop; fp8 out, 2x mode)
                    nc.vector.tensor_single_scalar(
                        s_g[:, b0 * NFREE : b1 * NFREE],
                        h_g[:, b0 * NFREE : b1 * NFREE],
                        1.0,
                        mybir.AluOpType.is_ge,
                    ).then_inc(s_act, 1)
                    act_incs += 1
                    continue
                # progress target: all slices up to b1 of this group done
                done_slices = dve_incs - (nsp - 1 - si)
                nc.scalar.wait_ge(s_dve, done_slices)
                if si == 0 and gi >= NRING:
                    # s-ring reuse: slot's previous store DMA must be done
                    nc.scalar.wait_ge(
                        s_dma[gi % NRING], 16 * dma_cnt[gi % NRING]
                    )
                nc.scalar.activation(
                    s_g[:, b0 * NFREE : b1 * NFREE],
                    h_g[:, b0 * NFREE : b1 * NFREE],
                    mybir.ActivationFunctionType.Sigmoid,
                    bias=sigb,
                    scale=2.0**100,
                ).then_inc(s_act, 1)
                act_incs += 1
            acts_after_group.append(act_incs)
            # store DMA for the whole group
            nc.sync.wait_ge(s_act, act_incs)
            lo = o_base * NFREE
            hi = (o_base + g_sz) * NFREE
            nc.sync.dma_start(
                out=out2[:, lo:hi], in_=s_g[:, : g_sz * NFREE]
            ).then_inc(s_dma[gi % NRING], 16)
            dma_cnt[gi % NRING] += 1
            o_base += g_sz

        # end of program: every store DMA landed, then reset semaphores so a
        # re-execution of the same NEFF starts from zero (mirrors Tile's
        # drain + barrier + RANGE_CLEAR epilogue)
        for j in range(NRING):
            nc.sync.wait_ge(s_dma[j], 16 * dma_cnt[j])
        nc.sync.drain()
        if RAW_EPILOGUE:
            nc.all_engine_barrier()
            nc.clear_and_free_semaphores([s_x, s_dve, s_act, *s_dma])
            nc.all_engine_barrier()

    nc.finalize()
    return nc


def _get_module(spike_engine: str, imm_coefs=None):
    raw = RAW and spike_engine == "act" and imm_coefs is not None
    if imm_coefs is not None:
        key = (spike_engine, SPIKE_DTYPE, raw,
               imm_coefs[0].tobytes(), imm_coefs[1].tobytes())
    else:
        key = (spike_engine, SPIKE_DTYPE, raw)
    if key not in _MODULE_CACHE:
        if raw:
            _MODULE_CACHE[key] = _build_module_raw(imm_coefs)
        else:
            _MODULE_CACHE[key] = _build_module(spike_engine, imm_coefs=imm_coefs)
    return _MODULE_CACHE[key]


def _prepare_inputs(inputs, enc_w, enc_b, bn_w, bn_b, bn_mean, bn_var):
    """Host-side marshalling: scalar folding + per-core shard/transpose."""
    x = np.ascontiguousarray(np.asarray(inputs, np.float32))
    w = np.asarray(enc_w, np.float32).reshape(_O)
    b = np.asarray(enc_b, np.float32).reshape(_O)
    bw = np.float64(np.asarray(bn_w).reshape(())[()])
    bb = np.float64(np.asarray(bn_b).reshape(())[()])
    bm = np.float64(np.asarray(bn_mean).reshape(())[()])
    bv = np.float64(np.asarray(bn_var).reshape(())[()])

    inv = bw / np.sqrt(bv + _EPS)
    beta = bb - bm * inv
    A = (inv * w.astype(np.float64) / _TAU).astype(np.float32)
    C = (((beta * w.astype(np.float64)) + b.astype(np.float64)) / _TAU).astype(
        np.float32
    )
    a_b = np.ascontiguousarray(np.broadcast_to(A, (_P, _O)))
    c_b = np.ascontiguousarray(np.broadcast_to(C, (_P, _O)))

    in_maps = []
    for core in range(_NC):
        xc = x[core * _BL : (core + 1) * _BL]          # [4, T, F]
        xt = np.ascontiguousarray(xc.transpose(0, 2, 1)).reshape(_BL * _F, _T)
        in_maps.append({"x_bft": xt, "a_coef": a_b, "c_coef": c_b})
    return in_maps


def _to_f32_spikes(v: np.ndarray) -> np.ndarray:
    """Device spike array -> f32 0.0/1.0 (exact: spike encodings are
    0x00 vs nonzero in every supported dtype; sigmoid emits +0.0 only)."""
    v = np.asarray(v)
    if v.dtype == np.float32:
        return v
    if v.itemsize == 1:
        return (v.view(np.uint8) != 0).astype(np.float32)
    if v.itemsize == 2:
        return (v.view(np.uint16) != 0).astype(np.float32)
    raise ValueError(f"unexpected spike dtype {v.dtype}")


def _unpack_core(spk_blocked: np.ndarray) -> np.ndarray:
    """[p=(b1,f), o, g, t] -> [b=2g+b1, o, f, t] (widened to f32)."""
    v = _to_f32_spikes(spk_blocked)
    v = v.reshape(2, _F, _O, _G, _T)                     # [b1, f, o, g, t]
    v = v.transpose(3, 0, 2, 1, 4)                       # [g, b1, o, f, t]
    return np.ascontiguousarray(v.reshape(_BL, _O, _F, _T))


def _run(in_maps, spike_engine=None, **spmd_kwargs):
    from concourse.bass_utils import run_bass_kernel_spmd

    eng = spike_engine or SPIKE_ENGINE
    imm_coefs = None
    if USE_IMM:
        imm_coefs = (in_maps[0]["a_coef"][0], in_maps[0]["c_coef"][0])
    nc = _get_module(eng, imm_coefs)
    return run_bass_kernel_spmd(nc, in_maps, core_ids=list(range(_NC)), **spmd_kwargs)


def kernel(inputs, enc_w, enc_b, bn_w, bn_b, bn_mean, bn_var):
    in_maps = _prepare_inputs(inputs, enc_w, enc_b, bn_w, bn_b, bn_mean, bn_var)
    res = _run(in_maps)
    out = np.concatenate([_unpack_core(r["spikes"]) for r in res.results], axis=0)
    return np.ascontiguousarray(out.astype(np.float32, copy=False))
